# revision 38
# baseline (speedup 1.0000x reference)
"""Trainium2 Bass kernel for nn_Attention_kv (dense transformer block).

Sharding: data-parallel over batch B=8 across the 8 NeuronCores — one batch
element per core, no collectives.

Key optimizations vs the straightforward per-element pipeline:

1. MASK COMPACTION. mask ~ Bernoulli(0.5) over the M=1024 sequence; in the
   reference, masked positions (a) contribute exactly 0 as attention keys
   (additive -10000 → exp underflows to exactly 0 in fp32), and (b) as
   queries produce a closed-form row — uniform attention over ALL keys,
   which by linearity is mean_rows(text_x) @ Wkv_v + bkv_v pushed through
   the ffn. The host gathers the ~512 valid positions — padded only to a
   multiple of 8 (partial PE tiles handle the ragged edge; queries/keys
   ride the free axis or a short lhsT, so no 128-padding is needed) — the
   device runs the whole pipeline on the ~0.54x-length sequence (~0.54x
   projections, ~0.29x attention), and the host scatters the result back,
   filling masked rows with the closed-form constant row (one [C]@[C,C]
   matvec per batch element, fp64 on host).

2. On device, only PAD positions need masking, done with zero
   vector-engine work:
   - q copyback fuses (+bias)*scale (tensor_scalar, same cost as bias add)
   - v/cv projections add their bias via a rank-1 PE matmul whose lhsT is
     the compacted-mask row -> pad v-rows are exactly 0
   - softmax denominators come from a mask-column (not ones) PE matmul ->
     pad keys are excluded exactly
   so attention needs NO masking ops at all: exp reads scores straight
   from PSUM; pad-query rows are discarded by the host scatter.
   When all biases are zero (true for the reference setup_inputs), the
   host selects a specialization that drops the rank-1 bias matmuls and
   bias copy-ops and folds 1/sqrt(C) into the attn1 normalization
   broadcast; the general-bias build remains available and correct.
   attn2's per-row 1/denominator column is produced by rank-1 (N=1) PE
   transpose matmuls (no DRAM bounce).

3. bfloat16 operands everywhere on the PE (fp32 PSUM accumulation;
   normalization stays fp32): halves DMA traffic and SBUF footprint.
   Measured end-to-end relative error ~5e-3 vs the fp32 reference
   (gate: 2e-2).

4. The host pre-transposes the gathered x/t (device consumes x^T
   directly), eliminating the PE-transpose phase; DMAs are ordered so the
   first projection group gates on ~1/4 of the head bytes, tiny constants
   ride the Activation-engine HWDGE queue, and every later weight is
   prefetched on a dedicated SBUF slot whose previous occupant dies
   exactly when the prefetch must start.

Measured (8 cores data-parallel, axon-tunneled TRN2): TimelineSim
111.5us vs 240.6us for the pre-session baseline; HW paired
marginal-per-body ~104us vs 263us for the baseline measured in the same
process (n_hi=101 replication; the axon tunnel adds +-30ms skewed
per-call noise, so only robust paired estimates are meaningful). HW
relative error 5.6e-3 (gate 2e-2). Note: GPSIMD cannot read PSUM on
real HW (the BIR verifier rejects it even though CoreSim/TimelineSim
accept it) — PSUM drains must stay on DVE/Act.

Per-core device pipeline (compacted seq mc≈552, dim C=768):
  qkv projection (q^T ((x@Wq)+b)*scale and k^T produced transposed
  [d, seq]; v natural with mask-gated bias)
  -> attn1: scores computed TRANSPOSED S^T[sk, sq]; max-free softmax;
     denominators via mask-column PE matmul; out^T accumulated across 6
     PSUM banks flash-style; normalization DEFERRED into the next
     phase's PSUM copyback
  -> cq projection -> kv projection (from t) -> attn2 -> ffn -> out.
"""

import sys

sys.path.insert(0, "/opt/trn_rl_repo")

from contextlib import ExitStack

import numpy as np
import ml_dtypes

import concourse.bass as bass
import concourse.mybir as mybir
import concourse.tile as tile
from concourse import bacc
from concourse.bass_utils import run_bass_kernel_spmd

BF16NP = ml_dtypes.bfloat16

P = 128
M = 1024  # full sequence length per batch element
C = 768  # model dim
KT = C // P  # 6 contraction tiles
SCALE = float(C) ** -0.5

F32 = mybir.dt.float32
F32R = mybir.dt.float32r
DT = mybir.dt.bfloat16
AL = mybir.AluOpType
AF = mybir.ActivationFunctionType

N_CORES = 8


def _tiles(mc):
    """Seq-tile heights: full 128s plus a ragged tail."""
    mct = (mc + P - 1) // P
    hs = [P] * (mct - 1) + [mc - P * (mct - 1)]
    return mct, hs


def _proj_natural(nc, lhs_src, w_rhs, dst, bias_row, mask_row, psum_pool, mc, tag):
    """dst[0:h, i, :] = (src @ W + mask⊗bias) rows [128i, 128i+h).
    bias_row=None skips the rank-1 bias (pad rows are 0 via zero x-cols).

    lhs_src: AP [P, KT, mc] (x^T layout) -- lhsT tiles [P, h]
    w_rhs: AP [P, KT, C] (weight) -- rhs tiles [P, chunk]
    bias_row: [1, C]; mask_row: [1, >=mc] (1=valid, 0=pad).
    The rank-1 mask⊗bias term zeroes pad rows entirely (bias included).
    """
    mct, hs = _tiles(mc)
    chunks = [(0, 512), (512, 256)]
    for i, h in enumerate(hs):
        pss = []
        for (off, w) in chunks:
            ps = psum_pool.tile([P, 512], F32, tag="st", name=f"ps_{tag}_{i}_{off}")
            for a in range(KT):
                nc.tensor.matmul(
                    ps[0:h, :w],
                    lhs_src[:, a, i * P : i * P + h],
                    w_rhs[:, a, off : off + w],
                    start=(a == 0),
                    stop=(bias_row is None and a == KT - 1),
                )
            if bias_row is not None:
                nc.tensor.matmul(
                    ps[0:h, :w],
                    mask_row[0:1, i * P : i * P + h],
                    bias_row[0:1, off : off + w],
                    start=False,
                    stop=True,
                )
            pss.append(ps)
        for (off, w), ps in zip(chunks, pss):
            nc.any.tensor_copy(out=dst[0:h, i, off : off + w], in_=ps[0:h, :w])


def _attention(nc, io, psum_pool, qT, kT, vn, outT, mask_col_r,
               ones_row_r, label, mc, recip_col=None, one_r=None):
    """outT[:, d, :] = (UNNORMALIZED attn numerator)^T, [P, KT, mc] bf16.

    qT must be PRE-SCALED (by 1/sqrt(C)); vn must have pad rows zeroed.
    Denominator = mask-column @ p (excludes pad keys exactly).
    Normalization is deferred to the consumer: returns per-chunk rbc
    broadcast tiles [P, fch] (1/rowsum along free sq) unless recip_col is
    given, in which case recip values are instead written into
    recip_col ([P, mct] column layout) and no bcast is made.
    """
    mct, hs = _tiles(mc)
    nch = 2
    fch = mc // nch  # mc is a multiple of 8 -> fch of 4
    rbcs = []
    recip_full = None
    if recip_col is not None:
        recip_full = io.tile([1, mc], F32, tag="recip_full",
                             name=f"recip_full_{label}", bufs=2)
    for c in range(nch):
        sq = slice(c * fch, (c + 1) * fch)
        # out^T accumulators: 6 banks
        pos = [
            psum_pool.tile([P, fch], F32, tag="po", name=f"po_{label}_{c}_{d}")
            for d in range(KT)
        ]
        p_tiles = []
        prev = None  # (j, h, p_j) pending out^T matmuls
        for j, h in enumerate(hs):
            st = psum_pool.tile([P, fch], F32, tag="st", name=f"st_{label}_{c}_{j}")
            for a in range(KT):
                nc.tensor.matmul(
                    st[0:h, :],
                    kT[:, a, j * P : j * P + h],
                    qT[:, a, sq],
                    start=(a == 0),
                    stop=(a == KT - 1),
                )
            pj = io.tile([P, fch], DT, tag="pp", name=f"p_{label}_{c}_{j}",
                         bufs=mct + 2)
            nc.scalar.activation(pj[0:h, :], st[0:h, :], AF.Exp)
            p_tiles.append(pj)
            if prev is not None:
                jj, hh, pprev = prev
                for d in range(KT):
                    nc.tensor.matmul(
                        pos[d][:],
                        vn[0:hh, jj, d * P : (d + 1) * P],
                        pprev[0:hh, :],
                        start=(jj == 0),
                        stop=False,
                    )
            prev = (j, h, pj)
        jj, hh, pprev = prev
        for d in range(KT):
            nc.tensor.matmul(
                pos[d][:],
                vn[0:hh, jj, d * P : (d + 1) * P],
                pprev[0:hh, :],
                start=(jj == 0),
                stop=True,
            )
        # UNNORMALIZED copyback first; nc.any lets the scheduler split
        # the drain across DVE/Act (GPSIMD cannot read PSUM) so the
        # reciprocal chain is not stuck behind six serial DVE copies
        for d in range(KT):
            nc.any.tensor_copy(out=outT[:, d, sq], in_=pos[d][:])
        # denominators: sum over VALID sk only, via mask-column matmuls
        rs = psum_pool.tile([P, fch], F32, tag="po", name=f"rs_{label}_{c}")
        for j, h in enumerate(hs):
            nc.tensor.matmul(
                rs[0:1, :],
                mask_col_r[0:h, j : j + 1],
                p_tiles[j][0:h, :],
                start=(j == 0),
                stop=(j == mct - 1),
            )
        if recip_col is None:
            recip = io.tile([1, fch], F32R, tag="recip", name=f"recip_{label}_{c}", bufs=2)
            with nc.allow_low_precision(reason="f32r recip feeds f32r bcast matmul"):
                nc.vector.reciprocal(recip[:], rs[0:1, :])
            # broadcast recip across partitions via K=1 matmul
            bc = psum_pool.tile([P, fch], F32, tag="po", name=f"bc_{label}_{c}")
            nc.tensor.matmul(bc[:], ones_row_r[:], recip[:], start=True, stop=True)
            rbc = io.tile([P, fch], F32, tag="rbc", name=f"rbc_{label}_{c}", bufs=2)
            nc.vector.tensor_copy(out=rbc[:], in_=bc[:])
            rbcs.append(rbc)
        else:
            nc.vector.reciprocal(recip_full[0:1, sq], rs[0:1, :])
    if recip_col is not None:
        return recip_full
    return rbcs


def build_nc(n_iters=1, mc=552, zero_bias=False):
    if isinstance(mc, tuple):
        mc, zero_bias = mc
    mct, hs = _tiles(mc)
    nch = 2
    fch = mc // nch
    LP = mct * P  # 128-padded mask length

    nc = bacc.Bacc(trn_type="TRN2", target_bir_lowering=False, debug=False)

    xT_d = nc.dram_tensor("xT", [C, mc], DT, kind="ExternalInput").ap()
    tT_d = nc.dram_tensor("tT", [C, mc], DT, kind="ExternalInput").ap()
    mask_d = nc.dram_tensor("mask", [1, LP], F32, kind="ExternalInput").ap()
    maskh_d = nc.dram_tensor("maskh", [1, LP], DT, kind="ExternalInput").ap()
    bqkvh_d = nc.dram_tensor("bqkvh", [1, 3 * C], DT, kind="ExternalInput").ap()
    bkvh_d = nc.dram_tensor("bkvh", [1, 2 * C], DT, kind="ExternalInput").ap()
    wqkv_d = nc.dram_tensor("Wqkv", [C, 3 * C], DT, kind="ExternalInput").ap()
    bqkv_d = nc.dram_tensor("bqkv", [1, 3 * C], F32, kind="ExternalInput").ap()
    wq_d = nc.dram_tensor("Wq", [C, C], DT, kind="ExternalInput").ap()
    bq_d = nc.dram_tensor("bq", [1, C], F32, kind="ExternalInput").ap()
    wkv_d = nc.dram_tensor("Wkv", [C, 2 * C], DT, kind="ExternalInput").ap()
    bkv_d = nc.dram_tensor("bkv", [1, 2 * C], F32, kind="ExternalInput").ap()
    wffn_d = nc.dram_tensor("Wffn", [C, C], DT, kind="ExternalInput").ap()
    bffn_d = nc.dram_tensor("bffn", [1, C], F32, kind="ExternalInput").ap()
    out_d = nc.dram_tensor("out", [mc, C], DT, kind="ExternalOutput").ap()

    xT_t = xT_d.rearrange("(a p) m -> p a m", p=P)  # [P, KT, mc]
    tT_t = tT_d.rearrange("(a p) m -> p a m", p=P)
    wqkv_t = wqkv_d.rearrange("(a p) n -> p a n", p=P)  # [P, KT, 3C]
    wq_t = wq_d.rearrange("(a p) n -> p a n", p=P)
    wkv_t = wkv_d.rearrange("(a p) n -> p a n", p=P)
    wffn_t = wffn_d.rearrange("(a p) n -> p a n", p=P)

    with tile.TileContext(nc) as tc, ExitStack() as ctx:
        const = ctx.enter_context(tc.tile_pool(name="const", bufs=1))
        acts = ctx.enter_context(tc.tile_pool(name="acts", bufs=1))
        wpool = ctx.enter_context(tc.tile_pool(name="wpool", bufs=1))
        io = ctx.enter_context(tc.tile_pool(name="io", bufs=1))
        psum_main = ctx.enter_context(tc.tile_pool(name="psum_main", bufs=2, space="PSUM"))
        psum_att = ctx.enter_context(tc.tile_pool(name="psum_att", bufs=6, space="PSUM"))

        # ---- head DMAs for body 0 (wq on SP queue, xT on Act queue:
        # concurrent rings, so the first projection group gates on
        # max(wq_slice, xT_half) instead of their sum) ----
        head0 = _issue_head(nc, acts, wpool, xT_t, wqkv_t, mc, 0)

        # ---- constants (all tiny: issued on the Act HWDGE queue behind
        # the xT halves) ----
        mask_t = const.tile([P, mct], F32, tag="mask_t", name="mask_t")
        nc.scalar.dma_start(mask_t[:], mask_d[0].rearrange("(a p) -> p a", p=P))
        mask_col_r = const.tile([P, mct], DT, tag="mask_col_r", name="mask_col_r")
        nc.vector.tensor_copy(out=mask_col_r[:], in_=mask_t[:])
        mask_row_r = const.tile([1, LP], DT, tag="mask_row_r", name="mask_row_r")
        nc.scalar.dma_start(mask_row_r[:], maskh_d[:])

        # attn1 normalization broadcast row: carries 1/sqrt(C) when biases
        # are all zero (folds the cq scale into the rbc multiply)
        ones_row_r = const.tile([1, P], F32R, tag="ones_row_r", name="ones_row_r")
        nc.gpsimd.memset(ones_row_r[:].bitcast(F32), SCALE if zero_bias else 1.0)
        one_r = const.tile([1, 1], F32, tag="one_r", name="one_r")
        nc.gpsimd.memset(one_r[:], 1.0)

        if zero_bias:
            bq_col = bk_col = bcq_col = bck_col = None
            bv_row = bcv_row = ffn_bias_bc = None
        else:
            # per-partition bias columns (d on partitions)
            bq_col = const.tile([P, KT], F32, tag="bq_col", name="bq_col")
            nc.scalar.dma_start(bq_col[:], bqkv_d[0, 0:C].rearrange("(a p) -> p a", p=P))
            bk_col = const.tile([P, KT], F32, tag="bk_col", name="bk_col")
            nc.scalar.dma_start(bk_col[:], bqkv_d[0, C : 2 * C].rearrange("(a p) -> p a", p=P))
            bcq_col = const.tile([P, KT], F32, tag="bcq_col", name="bcq_col")
            nc.scalar.dma_start(bcq_col[:], bq_d[0, :].rearrange("(a p) -> p a", p=P))
            bck_col = const.tile([P, KT], F32, tag="bck_col", name="bck_col")
            nc.scalar.dma_start(bck_col[:], bkv_d[0, 0:C].rearrange("(a p) -> p a", p=P))

            # bias rows (bf16) for the rank-1 mask⊗bias matmuls
            bv_row = const.tile([1, C], DT, tag="bv_row", name="bv_row")
            nc.scalar.dma_start(bv_row[:], bqkvh_d[0:1, 2 * C : 3 * C])
            bcv_row = const.tile([1, C], DT, tag="bcv_row", name="bcv_row")
            nc.scalar.dma_start(bcv_row[:], bkvh_d[0:1, C : 2 * C])

            # ffn bias broadcast (applied in the final stt epilogue)
            ffn_bias_bc = const.tile([P, C], F32, tag="bbc_f", name="ffnbias_bc")
            nc.scalar.dma_start(ffn_bias_bc[:], bffn_d[0:1, :].partition_broadcast(P))

        consts = dict(mask_col_r=mask_col_r, mask_row_r=mask_row_r,
                      ones_row_r=ones_row_r, bq_col=bq_col, bk_col=bk_col,
                      bcq_col=bcq_col, bck_col=bck_col, bv_row=bv_row,
                      bcv_row=bcv_row, ffn_bias_bc=ffn_bias_bc,
                      one_r=one_r, zero_bias=zero_bias)

        for _it in range(n_iters):
            head = head0 if _it == 0 else _issue_head(
                nc, acts, wpool, xT_t, wqkv_t, mc, _it
            )
            _body_iter(nc, tc, acts, wpool, io, psum_main, psum_att,
                       head, consts,
                       tT_t, wq_t, wkv_t, wffn_t, out_d, mc, _it)

    nc.compile()
    return nc


def _issue_head(nc, acts, wpool, xT_t, wqkv_t, mc, it):
    """Issue the head DMAs: wq/xT/wk interleaved so the first projection
    group gates on ~1/4 of the bytes, then vw."""
    H = C // 2
    fch = mc // 2
    xT = acts.tile([P, KT, mc], DT, tag="xT", name=f"xT_{it}", bufs=2)
    wq = wpool.tile([P, KT, C], DT, tag="ws1", name=f"w_q_{it}", bufs=1)
    wk = wpool.tile([P, KT, C], DT, tag="ws2", name=f"w_k_{it}", bufs=1)
    # first projection group gates on just wq[0:128] + xT[0:fch]
    nc.sync.dma_start(wq[:, :, 0:P], wqkv_t[:, :, 0:P])
    nc.scalar.dma_start(xT[:, :, 0:fch], xT_t[:, :, 0:fch])
    nc.sync.dma_start(wq[:, :, P : 3 * P], wqkv_t[:, :, P : 3 * P])
    nc.scalar.dma_start(xT[:, :, fch:mc], xT_t[:, :, fch:mc])
    nc.sync.dma_start(wq[:, :, 3 * P : C], wqkv_t[:, :, 3 * P : C])
    nc.sync.dma_start(wk[:, :, 0:H], wqkv_t[:, :, C : C + H])
    nc.sync.dma_start(wk[:, :, H:C], wqkv_t[:, :, C + H : 2 * C])
    vw = wpool.tile([P, KT, C], DT, tag="ws3", name=f"vw_qkv_{it}", bufs=1)
    nc.sync.dma_start(vw[:], wqkv_t[:, :, 2 * C : 3 * C])
    return dict(xT=xT, wq=wq, wk=wk, vw=vw)


def _body_iter(nc, tc, acts, wpool, io, psum_main, psum_att,
               head, consts,
               tT_t, wq_t, wkv_t, wffn_t, out_d, mc, it):
    mct, hs = _tiles(mc)
    nch = 2
    fch = mc // nch

    mask_col_r = consts["mask_col_r"]
    mask_row_r = consts["mask_row_r"]
    ones_row_r = consts["ones_row_r"]

    xT = head["xT"]
    qT = acts.tile([P, KT, mc], DT, tag="qT", name=f"qT_{it}")
    kTt = acts.tile([P, KT, mc], DT, tag="kT", name=f"kT_{it}")
    vn = acts.tile([P, mct, C], DT, tag="vn", name=f"vn_{it}")
    o1T = acts.tile([P, KT, mc], DT, tag="oT", name=f"o1T_{it}")

    # ---- qkv projection: q^T scaled+biased, k^T biased ----
    for part, (dst, bcol, w, do_scale) in enumerate(
        [(qT, consts["bq_col"], head["wq"], True),
         (kTt, consts["bk_col"], head["wk"], False)]
    ):
        for c in range(nch):
            for d in range(KT):
                ps = psum_main.tile([P, fch], F32, tag="st",
                                    name=f"ps_qk_{it}_{part}_{d}_{c}")
                for a in range(KT):
                    nc.tensor.matmul(
                        ps[:],
                        w[:, a, d * P : (d + 1) * P],
                        xT[:, a, c * fch : (c + 1) * fch],
                        start=(a == 0),
                        stop=(a == KT - 1),
                    )
                dd = dst[:, d, c * fch : (c + 1) * fch]
                if consts["zero_bias"]:
                    if do_scale:
                        nc.any.tensor_scalar_mul(dd, ps[:], SCALE)
                    else:
                        nc.any.tensor_copy(out=dd, in_=ps[:])
                elif do_scale:
                    nc.any.tensor_scalar(dd, ps[:], bcol[:, d : d + 1], SCALE,
                                         AL.add, AL.mult)
                else:
                    nc.any.tensor_scalar_add(dd, ps[:], bcol[:, d : d + 1])

    _proj_natural(nc, xT, head["vw"], vn, consts["bv_row"], mask_row_r,
                  psum_main, mc, f"v_{it}")

    # ---- prefetch: t^T + all remaining weights (the sync queue drains
    # each as its dedicated SBUF slot's previous occupant dies) ----
    tT = acts.tile([P, KT, mc], DT, tag="tT", name=f"tT_{it}")
    nc.scalar.dma_start(tT[:], tT_t[:])
    wqs = wpool.tile([P, KT, C], DT, tag="ws4", name=f"wq_sb_{it}", bufs=1)
    nc.sync.dma_start(wqs[:], wq_t[:])
    wks = wpool.tile([P, KT, C], DT, tag="ws1", name=f"wk_sb_{it}", bufs=1)
    nc.sync.dma_start(wks[:], wkv_t[:, :, 0:C])
    cvw = wpool.tile([P, KT, C], DT, tag="ws2", name=f"vw_kv_{it}", bufs=1)
    nc.sync.dma_start(cvw[:], wkv_t[:, :, C : 2 * C])
    wfs = wpool.tile([P, KT, C], DT, tag="ws3", name=f"wffn_sb_{it}", bufs=1)
    nc.sync.dma_start(wfs[:], wffn_t[:])

    # ---- attention 1 ----
    class _AttPsum:
        def tile(self, shape, dtype, tag, name):
            pool = psum_att if tag == "po" else psum_main
            return pool.tile(shape, dtype, tag=tag, name=name)

    att_psum = _AttPsum()
    rbcs1 = _attention(
        nc, io, att_psum, qT, kTt, vn, o1T, mask_col_r,
        ones_row_r, f"a1_{it}", mc,
    )

    # ---- cq projection (into qT slot): ((o1*rbc) + bq)*scale ----
    bcq_col = consts["bcq_col"]
    cqT = acts.tile([P, KT, mc], DT, tag="qT", name=f"cqT_{it}")
    for d in range(KT):
        for c in range(nch):
            ps = psum_main.tile([P, fch], F32, tag="st", name=f"ps_cq_{it}_{d}_{c}")
            for a in range(KT):
                nc.tensor.matmul(
                    ps[:],
                    wqs[:, a, d * P : (d + 1) * P],
                    o1T[:, a, c * fch : (c + 1) * fch],
                    start=(a == 0),
                    stop=(a == KT - 1),
                )
            dst = cqT[:, d, c * fch : (c + 1) * fch]
            # zero_bias: rbc already carries the 1/sqrt(C) factor
            nc.any.tensor_mul(out=dst, in0=ps[:], in1=rbcs1[c][:])
            if not consts["zero_bias"]:
                nc.any.tensor_scalar(dst, dst, bcq_col[:, d : d + 1], SCALE,
                                     AL.add, AL.mult)

    # ---- kv projection from t (into kT, vn slots) ----
    bck_col = consts["bck_col"]
    ckT = acts.tile([P, KT, mc], DT, tag="kT", name=f"ckT_{it}")
    for d in range(KT):
        for c in range(nch):
            ps = psum_main.tile([P, fch], F32, tag="st", name=f"ps_ck_{it}_{d}_{c}")
            for a in range(KT):
                nc.tensor.matmul(
                    ps[:],
                    wks[:, a, d * P : (d + 1) * P],
                    tT[:, a, c * fch : (c + 1) * fch],
                    start=(a == 0),
                    stop=(a == KT - 1),
                )
            if consts["zero_bias"]:
                nc.any.tensor_copy(
                    out=ckT[:, d, c * fch : (c + 1) * fch], in_=ps[:]
                )
            else:
                nc.any.tensor_scalar_add(
                    ckT[:, d, c * fch : (c + 1) * fch], ps[:],
                    bck_col[:, d : d + 1]
                )

    cvn = acts.tile([P, mct, C], DT, tag="vn", name=f"cvn_{it}")
    _proj_natural(nc, tT, cvw, cvn, consts["bcv_row"], mask_row_r,
                  psum_main, mc, f"cv_{it}")

    # ---- attention 2 (out2T into xT slot) ----
    o2T = acts.tile([P, KT, mc], DT, tag="xT", name=f"o2T_{it}", bufs=2)
    recip2_col = io.tile([P, mct], F32, tag="recip2_col", name=f"recip2_col_{it}", bufs=2)
    recip_full = _attention(
        nc, io, att_psum, cqT, ckT, cvn, o2T, mask_col_r,
        ones_row_r, f"a2_{it}", mc,
        recip_col=recip2_col, one_r=consts["one_r"],
    )

    # ---- ffn ----
    ffn_bias_bc = consts["ffn_bias_bc"]
    chunks = [(0, 512), (512, 256)]
    for i, h in enumerate(hs):
        pss = []
        for (off, w) in chunks:
            ps = psum_main.tile([P, 512], F32, tag="st", name=f"ps_f_{it}_{i}_{off}")
            for a in range(KT):
                nc.tensor.matmul(
                    ps[0:h, :w],
                    o2T[:, a, i * P : i * P + h],
                    wfs[:, a, off : off + w],
                    start=(a == 0),
                    stop=(a == KT - 1),
                )
            pss.append(ps)
        if i == 0:
            # transpose the attn2 recip row into column layout via rank-1
            # (N=1) matmuls; tile-0's matmul group above hides the DVE
            # latency of attn2's trailing copybacks + reciprocals
            for ti, th in enumerate(hs):
                tr = psum_att.tile([P, 1], F32, tag="po", name=f"tr_{it}_{ti}")
                nc.tensor.matmul(tr[0:th, 0:1],
                                 recip_full[0:1, ti * P : ti * P + th],
                                 consts["one_r"][:], start=True, stop=True)
                nc.any.tensor_copy(out=recip2_col[0:th, ti : ti + 1],
                                   in_=tr[0:th, 0:1])
        fin = io.tile([P, C], DT, tag="fin", name=f"fin_{it}_{i}", bufs=2)
        for (off, w), ps in zip(chunks, pss):
            if consts["zero_bias"]:
                nc.vector.tensor_scalar_mul(
                    fin[0:h, off : off + w], ps[0:h, :w],
                    recip2_col[0:h, i : i + 1],
                )
            else:
                nc.vector.scalar_tensor_tensor(
                    out=fin[0:h, off : off + w],
                    in0=ps[0:h, :w],
                    scalar=recip2_col[0:h, i : i + 1],
                    in1=ffn_bias_bc[0:h, off : off + w],
                    op0=AL.mult,
                    op1=AL.add,
                )
        nc.scalar.dma_start(out_d[i * P : i * P + h, :], fin[0:h, :])


_NC_CACHE = {}


def _get_nc(mc, zero_bias=False):
    if isinstance(mc, tuple):
        mc, zero_bias = mc
    key = (mc, zero_bias)
    if key not in _NC_CACHE:
        _NC_CACHE[key] = build_nc(mc=mc, zero_bias=zero_bias)
    return _NC_CACHE[key]


def prep_inputs(layout_x, text_x, mask, Wqkv, bqkv, Wq, bq, Wkv, bkv, Wffn, bffn):
    """Host-side compaction. Returns (in_maps, metas, mc)."""
    layout_x = np.ascontiguousarray(np.asarray(layout_x, dtype=np.float32))
    text_x = np.ascontiguousarray(np.asarray(text_x, dtype=np.float32))
    mask = np.ascontiguousarray(np.asarray(mask, dtype=np.float32))
    Wqkv = np.ascontiguousarray(np.asarray(Wqkv, dtype=np.float32))
    bqkv = np.ascontiguousarray(np.asarray(bqkv, dtype=np.float32)).reshape(1, 3 * C)
    Wq = np.ascontiguousarray(np.asarray(Wq, dtype=np.float32))
    bq = np.ascontiguousarray(np.asarray(bq, dtype=np.float32)).reshape(1, C)
    Wkv = np.ascontiguousarray(np.asarray(Wkv, dtype=np.float32))
    bkv = np.ascontiguousarray(np.asarray(bkv, dtype=np.float32)).reshape(1, 2 * C)
    Wffn = np.ascontiguousarray(np.asarray(Wffn, dtype=np.float32))
    bffn = np.ascontiguousarray(np.asarray(bffn, dtype=np.float32)).reshape(1, C)

    B = layout_x.shape[0]
    assert B == N_CORES
    zero_bias = not (bqkv.any() or bq.any() or bkv.any() or bffn.any())
    idxs = [np.nonzero(mask[b] != 0.0)[0] for b in range(B)]
    max_n = max(len(ix) for ix in idxs)
    # pad to a multiple of 8 (ragged PE tiles handle the rest); keep >= 16
    mc = max(16, -(-max_n // 8) * 8)
    mct = (mc + P - 1) // P
    LP = mct * P

    Wqkv_h = Wqkv.astype(BF16NP)
    Wq_h = Wq.astype(BF16NP)
    Wkv_h = Wkv.astype(BF16NP)
    Wffn_h = Wffn.astype(BF16NP)

    # closed-form constant row for fully-masked queries (fp64 on host):
    # uniform attention over ALL keys -> mean(cv) -> ffn
    mean_t = text_x.astype(np.float64).mean(axis=1)  # [B, C]
    mean_cv = mean_t @ Wkv[:, C:].astype(np.float64) + bkv[0, C:].astype(np.float64)
    mrows = (mean_cv @ Wffn.astype(np.float64) + bffn[0].astype(np.float64)).astype(
        np.float32
    )  # [B, C]

    in_maps, metas = [], []
    for b in range(B):
        ix = idxs[b]
        n = len(ix)
        xT = np.zeros((C, mc), dtype=BF16NP)
        tT = np.zeros((C, mc), dtype=BF16NP)
        xT[:, :n] = layout_x[b].T[:, ix]
        tT[:, :n] = text_x[b].T[:, ix]
        mk = np.zeros((1, LP), dtype=np.float32)
        mk[0, :n] = 1.0
        in_maps.append(
            {
                "xT": xT,
                "tT": tT,
                "mask": mk,
                "maskh": mk.astype(BF16NP),
                "Wqkv": Wqkv_h,
                "bqkv": bqkv,
                "bqkvh": bqkv.astype(BF16NP),
                "Wq": Wq_h,
                "bq": bq,
                "Wkv": Wkv_h,
                "bkv": bkv,
                "bkvh": bkv.astype(BF16NP),
                "Wffn": Wffn_h,
                "bffn": bffn,
            }
        )
        metas.append((ix, n, mrows[b]))
    return in_maps, metas, (mc, zero_bias)


def postprocess(outs, metas):
    """outs: list of [mc, C] device outputs (bf16). Scatter to [B, M, C]."""
    B = len(outs)
    full = np.empty((B, M, C), dtype=np.float32)
    for b in range(B):
        ix, n, mrow = metas[b]
        full[b, :, :] = mrow[None, :]
        full[b, ix, :] = np.asarray(outs[b][:n], dtype=np.float32)
    return full


def kernel(layout_x, text_x, mask, Wqkv, bqkv, Wq, bq, Wkv, bkv, Wffn, bffn):
    in_maps, metas, mc = prep_inputs(
        layout_x, text_x, mask, Wqkv, bqkv, Wq, bq, Wkv, bkv, Wffn, bffn
    )
    nc = _get_nc(mc)
    res = run_bass_kernel_spmd(nc, in_maps, core_ids=list(range(N_CORES)))
    return postprocess(
        [res.results[b]["out"] for b in range(N_CORES)], metas
    )
